# revision 18
# baseline (speedup 1.0000x reference)
"""GAT layer kernel for Trainium2, data-parallel over batch across 8 NeuronCores.

Per batch element b (one core each):
    hp  = h @ W_proj + b_proj                      # [N, D]
    s   = hp @ w_src ; t = hp @ w_dst              # [N]
    e   = relu(s[:,None] + t[None,:] + b_att)      # [N, N]
    att = exp(e) * a ; att /= att.sum(-1, keepdim) # [N, N]
    out = att @ hp + hp                            # [N, D]

Key identities:
  exp(relu(x)) == max(exp(x), 1)     -> relu becomes a max against 1.0
  exp(s_i+t_j) == exp(s_i)*exp(t_j)  -> the NxN exp becomes a rank-1 outer
                                        product u_i * v_j: no per-block ACT
                                        pass, just a DVE tensor_scalar.

Per 128-row block of the score matrix (4 blocks per super-block):
  - SWDGE cast-DMA loads the a-block f32->bf16
  - DVE tensor_scalar:        zc = max(v_full * u_i, 1)     (4x bf16 mode)
  - DVE scalar_tensor_tensor: P = zc * a, accum -> rowsum   (2x bf16 mode)
  - one batched xbar DMA-transpose P -> P^T (blocked [128,16,128] layout)
Per super-block (512 rows):
  - PE: 16 bf16 matmuls, lhsT=hp chunk (one LDWEIGHTS each), rhs = 512-wide
    P^T slab, accumulated into a [128,512] PSUM bank: outT = (P @ hp).T
  - finalize: ACT copy psum->sbuf, PE transposes back to [i,d], ACT applies
    1/rowsum (per-partition scale), DVE adds the hp residual.
"""

import os
import sys

for _p in ("/opt/trn_rl_repo", "/root/.axon_site/_ro/trn_rl_repo"):
    if _p not in sys.path and os.path.isdir(_p):
        sys.path.append(_p)

import numpy as np
from contextlib import ExitStack

import concourse.bass as bass
import concourse.bacc as bacc
import concourse.tile as tile
from concourse import masks, mybir
from concourse.bass_utils import run_bass_kernel_spmd

F32 = mybir.dt.float32
BF16 = mybir.dt.bfloat16

B, N, D = 8, 2048, 128
P = 128           # partitions
NT = N // P       # 16 row/col blocks
SB = 4            # row-blocks per super-block
NSB = NT // SB    # super-blocks
N_CORES = 8


def _build_kernel(ctx: ExitStack, tc: tile.TileContext, io: dict):
    nc = tc.nc
    a = io["a"]            # [N, N] f32 dram
    h = io["h"]            # [N, D] f32 dram
    W = io["W_proj"]       # [D, D] f32 dram
    b_proj = io["b_proj"]  # [D, 1] f32 dram
    w_sd = io["w_sd"]      # [D, 2] f32 dram: [w_src | w_dst]
    b_att = io["b_att"]    # [1, 1] f32 dram
    out = io["out"]        # [N, D] f32 dram

    cst = ctx.enter_context(tc.tile_pool(name="cst", bufs=1))
    sps = ctx.enter_context(tc.tile_pool(name="sps", bufs=2, space="PSUM"))
    a_pool = ctx.enter_context(tc.tile_pool(name="a", bufs=1))

    # ---- identity (gpsimd) first, then a-prefetches keep the Pool/SWDGE
    # stream flowing during the whole setup phase ----
    ident = cst.tile([P, P], F32)
    masks.make_identity(nc, ident[:])

    # a-loads are emitted PREFETCH blocks ahead of their consumer so their
    # DMA-completion semaphore ticks precede the transposes' — otherwise the
    # shared completion lanes create false waits on the previous transpose.
    PREFETCH = 3
    a_tiles = {}

    def emit_load(r):
        if r < NT:
            a_t = a_pool.tile([P, N], BF16, tag=f"a{r % 6}")
            nc.gpsimd.dma_start(a_t[:], a[r * P:(r + 1) * P, :])  # cast DMA
            a_tiles[r] = a_t

    for _r in range(PREFETCH):
        emit_load(_r)

    W_sb = cst.tile([P, D], F32)
    nc.sync.dma_start(W_sb[:], W[:])
    bp_col = cst.tile([P, 1], F32)
    nc.sync.dma_start(bp_col[:], b_proj[:])
    # [w_src | 0...0 | w_dst] with w_dst in column 32: matmul output rows land
    # at partitions 0 and 32 (engine APs may only start at multiples of 32).
    wsd_sb = cst.tile([P, 33], F32)
    nc.vector.memset(wsd_sb[:], 0.0)
    nc.sync.dma_start(wsd_sb[:, 0:1], w_sd[:, 0:1])
    nc.sync.dma_start(wsd_sb[:, 32:33], w_sd[:, 1:2])
    ba_sb = cst.tile([1, 1], F32)
    nc.sync.dma_start(ba_sb[:], b_att[:])

    # ---- h natural tiles: [p, r, d] with h[r*128+p, d] ----
    h_sb = cst.tile([P, NT, D], F32)
    nc.sync.dma_start(h_sb[:], h.rearrange("(r p) d -> p r d", p=P))

    # ---- hT [in, n] via PE transposes ----
    hT = cst.tile([P, N], F32)
    for r in range(NT):
        ps = sps.tile([P, 512], F32, tag="sps")
        nc.tensor.matmul(ps[:, :P], h_sb[:, r, :], ident[:], is_transpose=True)
        nc.scalar.copy(hT[:, r * P:(r + 1) * P], ps[:, :P])

    # ---- hpT [d, n] = (h @ W + b).T : lhsT=W [in,d], rhs=hT [in,n] ----
    hpT = cst.tile([P, N], F32)
    for s4 in range(4):
        sl = slice(s4 * 512, (s4 + 1) * 512)
        ps = sps.tile([P, 512], F32, tag="sps")
        nc.tensor.matmul(ps[:], W_sb[:], hT[:, sl])
        nc.scalar.activation(hpT[:, sl], ps[:],
                             mybir.ActivationFunctionType.Identity,
                             bias=bp_col[:], scale=1.0)

    # ---- hp natural (f32 for the residual add, bf16 for the matmul lhsT) ----
    hp_nat = cst.tile([P, NT, D], F32)
    hp_b16 = cst.tile([P, NT, D], BF16)
    for r in range(NT):
        ps = sps.tile([P, 512], F32, tag="sps")
        nc.tensor.matmul(ps[:, :P], hpT[:, r * P:(r + 1) * P], ident[:],
                         is_transpose=True)
        nc.scalar.copy(hp_nat[:, r, :], ps[:, :P])
        nc.vector.tensor_copy(hp_b16[:, r, :], hp_nat[:, r, :])

    # ---- t_row [1, n] = hp @ w_dst + b_att, replicated across partitions ----
    t_row = cst.tile([1, N], F32)
    for s4 in range(4):
        sl = slice(s4 * 512, (s4 + 1) * 512)
        ps = sps.tile([P, 512], F32, tag="sps")
        nc.tensor.matmul(ps[:33, :], wsd_sb[:], hpT[:, sl])
        nc.scalar.activation(t_row[:, sl], ps[32:33, :],
                             mybir.ActivationFunctionType.Identity,
                             bias=ba_sb[:], scale=1.0)
    t_full = cst.tile([P, N], F32)
    nc.sync.dma_start(t_full[0:1, :], t_row[:])
    reps = 1
    while reps < P:
        n2 = min(reps, P - reps)
        nc.sync.dma_start(t_full[reps:reps + n2, :], t_full[0:n2, :])
        reps += n2

    # ---- s_col [p, r] = s[r*128+p], s = hp @ w_src ----
    s_col = cst.tile([P, NT], F32)
    s_ps = sps.tile([P, 512], F32, tag="sps")
    for r in range(NT):
        nc.tensor.matmul(s_ps[:, r:r + 1], hpT[:, r * P:(r + 1) * P],
                         wsd_sb[:, 0:1])
    nc.scalar.copy(s_col[:], s_ps[:, :NT])

    # ---- main loop pools ----
    HB = NT // 2          # 8 row-blocks per half
    z_pool = ctx.enter_context(tc.tile_pool(name="z", bufs=1))
    pb_pool = ctx.enter_context(tc.tile_pool(name="pb", bufs=1))
    pbth_pool = ctx.enter_context(tc.tile_pool(name="pbth", bufs=1))
    rs_pool = ctx.enter_context(tc.tile_pool(name="rs", bufs=1))
    otsb_pool = ctx.enter_context(tc.tile_pool(name="otsb", bufs=1))
    osb_pool = ctx.enter_context(tc.tile_pool(name="osb", bufs=1))
    ops_pool = ctx.enter_context(tc.tile_pool(name="ops", bufs=1, space="PSUM"))
    tps_pool = ctx.enter_context(tc.tile_pool(name="tps", bufs=1, space="PSUM"))

    out_stage = cst.tile([P, NT, D], F32)

    # Finalize (normalize + residual) lagged one half so in-order engine
    # streams never stall on the current half's matmul groups.
    pending = []

    def finalize(o_ps_pair, rsums, hh):
        for k in range(2):
            oT_sb = otsb_pool.tile([P, 4 * P], F32, tag=f"ot{(2 * hh + k) % 2}")
            nc.scalar.copy(oT_sb[:], o_ps_pair[k][:])
            tp = tps_pool.tile([P, 4 * P], F32)
            for u in range(4):
                r = 8 * hh + 4 * k + u
                usl = slice(u * P, (u + 1) * P)
                nc.tensor.matmul(tp[:, usl], oT_sb[:, usl], ident[:],
                                 is_transpose=True)
                rinv = rs_pool.tile([P, 1], F32, tag=f"ri{u % 2}")
                nc.vector.reciprocal(rinv[:], rsums[4 * k + u][:])
                o_sb = osb_pool.tile([P, D], F32, tag=f"os{u % 4}")
                nc.scalar.activation(o_sb[:], tp[:, usl],
                                     mybir.ActivationFunctionType.Copy,
                                     scale=rinv[:])
                nc.vector.tensor_add(out_stage[:, r, :], o_sb[:],
                                     hp_nat[:, r, :])

    for hh in range(2):
        pbT_h = pbth_pool.tile([P, NT, HB * P], BF16, tag=f"pt{hh % 2}")
        rsums = []
        for u in range(HB):
            r = HB * hh + u
            emit_load(r + PREFETCH)
            a_t = a_tiles.pop(r)

            z_t = z_pool.tile([P, N], BF16, tag=f"z{r % 3}")
            nc.scalar.activation(z_t[:], t_full[:],
                                 mybir.ActivationFunctionType.Exp,
                                 bias=s_col[:, r:r + 1], scale=1.0)

            pb_t = pb_pool.tile([P, N], BF16, tag=f"pb{r % 3}")
            rsum = rs_pool.tile([P, 1], F32, tag=f"rs{r}")
            nc.vector.scalar_tensor_tensor(pb_t[:], z_t[:], 1.0, a_t[:],
                                           mybir.AluOpType.max,
                                           mybir.AluOpType.mult,
                                           accum_out=rsum[:])
            rsums.append(rsum)

            nc.sync.dma_start_transpose(
                out=pbT_h[:, :, u * P:(u + 1) * P], in_=pb_t[:])

        # Dense matmul phase: one LDWEIGHTS per chunk feeds two 512-wide
        # matmuls (the two quads of this half) back-to-back.
        o_a = ops_pool.tile([P, 4 * P], F32, tag=f"oa{hh}")
        o_b = ops_pool.tile([P, 4 * P], F32, tag=f"ob{hh}")
        for c in range(NT):
            nc.tensor.matmul(o_a[:], hp_b16[:, c, :], pbT_h[:, c, 0:4 * P],
                             start=(c == 0), stop=(c == NT - 1))
            nc.tensor.matmul(o_b[:], hp_b16[:, c, :], pbT_h[:, c, 4 * P:8 * P],
                             start=(c == 0), stop=(c == NT - 1))

        pending.append(((o_a, o_b), rsums, hh))

    # All finalize work runs as one batched tail: interleaving it into the
    # steady loop couples the in-order ACT/DVE streams and serializes blocks.
    for item in pending:
        finalize(*item)

    nc.sync.dma_start(out.rearrange("(r p) d -> p r d", p=P), out_stage[:])


_CACHE = {}


def _get_compiled():
    if "nc" in _CACHE:
        return _CACHE["nc"], _CACHE["names"]

    nc = bacc.Bacc("TRN2", target_bir_lowering=False, debug=False)
    io = {}
    io["a"] = nc.dram_tensor("a", [N, N], F32, kind="ExternalInput").ap()
    io["h"] = nc.dram_tensor("h", [N, D], F32, kind="ExternalInput").ap()
    io["W_proj"] = nc.dram_tensor("W_proj", [D, D], F32, kind="ExternalInput").ap()
    io["b_proj"] = nc.dram_tensor("b_proj", [D, 1], F32, kind="ExternalInput").ap()
    io["w_sd"] = nc.dram_tensor("w_sd", [D, 2], F32, kind="ExternalInput").ap()
    io["b_att"] = nc.dram_tensor("b_att", [1, 1], F32, kind="ExternalInput").ap()
    io["out"] = nc.dram_tensor("out", [N, D], F32, kind="ExternalOutput").ap()

    with tile.TileContext(nc) as tc:
        with ExitStack() as ctx:
            _build_kernel(ctx, tc, io)
    nc.compile()

    _CACHE["nc"] = nc
    _CACHE["names"] = list(io.keys())
    return nc, _CACHE["names"]


def _make_in_maps(a, h, W_proj, b_proj, w_att, b_att):
    a = np.ascontiguousarray(a, dtype=np.float32)
    h = np.ascontiguousarray(h, dtype=np.float32)
    W_proj = np.ascontiguousarray(W_proj, dtype=np.float32)
    b_proj = np.ascontiguousarray(b_proj, dtype=np.float32).reshape(D, 1)
    w_att = np.ascontiguousarray(w_att, dtype=np.float32)
    w_sd = np.stack([w_att[:D], w_att[D:]], axis=1).copy()  # [D, 2]
    b_att = np.asarray(b_att, dtype=np.float32).reshape(1, 1).copy()

    in_maps = []
    for c in range(N_CORES):
        in_maps.append({
            "a": a[c], "h": h[c], "W_proj": W_proj, "b_proj": b_proj,
            "w_sd": w_sd, "b_att": b_att,
        })
    return in_maps


def _get_executable():
    """Build (once) a sharded PJRT callable for the compiled Bass module.

    Mirrors concourse.bass2jax.run_bass_via_pjrt but keeps the jitted
    function so repeated calls don't retrace/recompile.
    """
    if "exe" in _CACHE:
        return _CACHE["exe"]

    import jax
    from jax.sharding import Mesh, PartitionSpec
    from jax.experimental.shard_map import shard_map
    from concourse import bass2jax, mybir as _mybir

    nc, _ = _get_compiled()
    bass2jax.install_neuronx_cc_hook()

    partition_name = (nc.partition_id_tensor.name
                      if nc.partition_id_tensor else None)
    in_names, out_names, out_avals, zero_outs = [], [], [], []
    for alloc in nc.m.functions[0].allocations:
        if not isinstance(alloc, _mybir.MemoryLocationSet):
            continue
        name = alloc.memorylocations[0].name
        if alloc.kind == "ExternalInput":
            if name != partition_name:
                in_names.append(name)
        elif alloc.kind == "ExternalOutput":
            shape = tuple(alloc.tensor_shape)
            dtype = _mybir.dt.np(alloc.dtype)
            out_names.append(name)
            out_avals.append(jax.core.ShapedArray(shape, dtype))
            zero_outs.append(np.zeros(shape, dtype))
    n_params = len(in_names)
    n_outs = len(out_avals)
    all_in_names = in_names + out_names + (
        [partition_name] if partition_name else [])
    donate = tuple(range(n_params, n_params + n_outs))

    def _body(*args):
        operands = list(args)
        if partition_name is not None:
            operands.append(bass2jax.partition_id_tensor())
        outs = bass2jax._bass_exec_p.bind(
            *operands,
            out_avals=tuple(out_avals),
            in_names=tuple(all_in_names),
            out_names=tuple(out_names),
            lowering_input_output_aliases=(),
            sim_require_finite=True,
            sim_require_nnan=True,
            nc=nc,
        )
        return tuple(outs)

    devices = jax.devices()[:N_CORES]
    mesh = Mesh(np.asarray(devices), ("core",))
    in_specs = (PartitionSpec("core"),) * (n_params + n_outs)
    out_specs = (PartitionSpec("core"),) * n_outs
    fn = jax.jit(
        shard_map(_body, mesh=mesh, in_specs=in_specs, out_specs=out_specs,
                  check_rep=False),
        donate_argnums=donate, keep_unused=True,
    )
    exe = {
        "fn": fn, "mesh": mesh, "in_names": in_names,
        "out_names": out_names, "out_avals": out_avals,
        "zero_outs": zero_outs, "n_params": n_params,
    }
    _CACHE["exe"] = exe
    return exe


def _concat_inputs(exe, in_maps):
    return [
        np.concatenate([np.asarray(in_maps[c][name])
                        for c in range(N_CORES)], axis=0)
        for name in exe["in_names"]
    ]


def _concat_zeros(exe):
    return [np.zeros((N_CORES * z.shape[0], *z.shape[1:]), z.dtype)
            for z in exe["zero_outs"]]


def kernel(a, h, W_proj, b_proj, w_att, b_att):
    exe = _get_executable()
    in_maps = _make_in_maps(a, h, W_proj, b_proj, w_att, b_att)
    out_arrs = exe["fn"](*_concat_inputs(exe, in_maps), *_concat_zeros(exe))
    i = exe["out_names"].index("out")
    return np.asarray(out_arrs[i]).reshape(N_CORES, N, D)


if __name__ == "__main__":
    rng = np.random.default_rng(0)
    a = rng.random((B, N, N), dtype=np.float32)
    h = rng.standard_normal((B, N, D), dtype=np.float32)
    W_proj = (rng.standard_normal((D, D)) / np.sqrt(D)).astype(np.float32)
    b_proj = (rng.standard_normal(D) * 0.01).astype(np.float32)
    w_att = (rng.standard_normal(2 * D) / np.sqrt(2 * D)).astype(np.float32)
    b_att = np.float32(rng.standard_normal() * 0.01)

    got = kernel(a=a, h=h, W_proj=W_proj, b_proj=b_proj, w_att=w_att,
                 b_att=b_att)

    hp = h @ W_proj + b_proj
    s = hp @ w_att[:D]
    t = hp @ w_att[D:]
    e = np.maximum(s[:, :, None] + t[:, None, :] + b_att, 0.0)
    att = np.exp(e) * a
    att = att / att.sum(-1, keepdims=True)
    ref = att @ hp + hp

    err = np.abs(got - ref).max() / np.abs(ref).max()
    print("rel err:", err)
